# revision 4
# baseline (speedup 1.0000x reference)
"""Multi-head attention (B=2, S=2048, E=1024, H=16, D=64) with additive bias
(alibi + mask) on 8 Trainium2 NeuronCores.

Sharding: 2 heads per core (head-parallel; batch kept whole per core).
Core c handles global heads (c, 15-c). Each core:
  - projects q/k/v for its 2 heads (feature-major fp16 matmuls),
  - computes attention transposed (logitsT[k,q]) so PV needs no transpose,
  - applies bias multiplicatively: softmax numerator = exp(logits) * exp(bias)
    where exp(bias) is precomputed on host in fp16 (zero where masked),
  - accumulates PV with an appended ones-column => row sums come for free,
  - computes its slice of the output projection; host sums partials.
Blocks of exp(bias) that are entirely zero (e.g. above the causal diagonal,
or beyond the fp16-underflow alibi band) are skipped; the skip set is the
union over all 16 heads so the SPMD program is identical on every core.
"""

import sys

if '/opt/trn_rl_repo' not in sys.path:
    sys.path.insert(0, '/opt/trn_rl_repo')
import os

os.environ.setdefault('MYCRO_LOCAL_CACHE', '1')

import numpy as np

B, S, E, H = 2, 2048, 1024, 16
D = E // H            # 64
NCORES = 8
BS = B * S            # 4096 tokens
KT = S // 128         # 16 k tiles per batch
QT = S // 512         # 4 q tiles per batch
F16 = np.float16

_cache = {}


def _build(nc, blocks, widths, qstart):
    """blocks: {qt: sorted list of kt}; widths/qstart: per-kt bias stripe window."""
    import concourse.bass as bass
    import concourse.mybir as mybir
    from concourse.tile import TileContext

    dt = mybir.dt
    f16, f32 = dt.float16, dt.float32
    AF = mybir.ActivationFunctionType

    nblocks = sum(len(v) for v in blocks.values())
    blk_index = {}
    for qt in range(QT):
        for kt in blocks[qt]:
            blk_index[(qt, kt)] = len(blk_index)
    all_kts = sorted({kt for v in blocks.values() for kt in v})

    xq = nc.dram_tensor('xq', (E, BS), f16, kind='ExternalInput')
    xk = nc.dram_tensor('xk', (E, BS), f16, kind='ExternalInput')
    xv = nc.dram_tensor('xv', (E, BS), f16, kind='ExternalInput')
    wqT = nc.dram_tensor('wqT', (E, 128), f16, kind='ExternalInput')
    wkT = nc.dram_tensor('wkT', (E, 128), f16, kind='ExternalInput')
    wvT = nc.dram_tensor('wvT', (E, 128), f16, kind='ExternalInput')
    wdT = nc.dram_tensor('wdT', (128, E), f16, kind='ExternalInput')
    ebias = nc.dram_tensor('ebias', (2, S, S), f16, kind='ExternalInput')
    attn_out = nc.dram_tensor('attn_out', (B, max(nblocks, 1), 128, 512), f16,
                              kind='ExternalOutput')
    partial = nc.dram_tensor('partial', (E, BS), f16, kind='ExternalOutput')

    with TileContext(nc) as tc:
        with (
            tc.tile_pool(name='wpool', bufs=1) as wpool,
            tc.tile_pool(name='xpool', bufs=3) as xpool,
            tc.tile_pool(name='hpool', bufs=1) as hpool,
            tc.tile_pool(name='vaug', bufs=1) as vaugp,
            tc.tile_pool(name='ebias', bufs=1) as ebp,
            tc.tile_pool(name='spool', bufs=4) as spool,
            tc.tile_pool(name='rpool', bufs=2) as rpool,
            tc.tile_pool(name='ppool', bufs=2) as ppool,
            tc.tile_pool(name='psmm', bufs=2, space='PSUM') as psmm,
            tc.tile_pool(name='pslg', bufs=2, space='PSUM') as pslg,
            tc.tile_pool(name='pspv', bufs=2, space='PSUM') as pspv,
        ):
            # ---- weights to SBUF (chunk c at cols [c*128,(c+1)*128)) ----
            w_sb = {}
            for name, wdram in (('q', wqT), ('k', wkT), ('v', wvT)):
                t = wpool.tile([128, E], f16, tag=f'w{name}')
                nc.sync.dma_start(t[:].rearrange('p (c m) -> p c m', m=128),
                                  wdram.rearrange('(c p) m -> p c m', p=128))
                w_sb[name] = t
            wd_sb = wpool.tile([128, E], f16, tag='wd')
            nc.sync.dma_start(wd_sb[:], wdT[:, :])

            # ---- projections: feature-major qhT/khT/vhT [128, BS] fp16 ----
            heads_sb = {}
            for name, xdram in (('q', xq), ('k', xk), ('v', xv)):
                dst = hpool.tile([128, BS], f16, tag=f'h{name}')
                heads_sb[name] = dst
                for tp in range(BS // 1024):
                    ps0 = psmm.tile([128, 512], f32, tag='mm')
                    ps1 = psmm.tile([128, 512], f32, tag='mm')
                    for c in range(E // 128):
                        xt = xpool.tile([128, 1024], f16, tag='xt')
                        nc.sync.dma_start(
                            xt[:], xdram[c * 128:(c + 1) * 128,
                                         tp * 1024:(tp + 1) * 1024])
                        st, sp = c == 0, c == E // 128 - 1
                        lhsT = w_sb[name][:, c * 128:(c + 1) * 128]
                        nc.tensor.matmul(ps0[:], lhsT, xt[:, 0:512],
                                         start=st, stop=sp)
                        nc.tensor.matmul(ps1[:], lhsT, xt[:, 512:1024],
                                         start=st, stop=sp)
                    nc.scalar.copy(dst[:, tp * 1024:tp * 1024 + 512], ps0[:])
                    nc.scalar.copy(dst[:, tp * 1024 + 512:tp * 1024 + 1024], ps1[:])

            # ---- vh_aug tiles: [128k, 64+1 | pad | 64+1] per (b, kt) ----
            vaug = {}
            for b in range(B):
                for kt in all_kts:
                    t = vaugp.tile([128, 146], f16, tag=f'va{b}_{kt}')
                    vaug[(b, kt)] = t
                    col = b * S + kt * 128
                    nc.sync.dma_start_transpose(
                        t[:, 0:64], heads_sb['v'][0:64, col:col + 128])
                    nc.sync.dma_start_transpose(
                        t[:, 80:144], heads_sb['v'][64:128, col:col + 128])
                    nc.vector.memset(t[:, 64:65], 1.0)
                    nc.vector.memset(t[:, 144:145], 1.0)

            # ---- attention, one local head at a time ----
            out_sb = hpool.tile([128, BS], f16, tag='out')
            for h in range(2):
                hb = h * 64
                ebt = {}
                for kt in all_kts:
                    w = widths[kt]
                    t = ebp.tile([128, w], f16, tag=f'eb{kt}')
                    ebt[kt] = t
                    nc.sync.dma_start(
                        t[:], ebias[h, kt * 128:(kt + 1) * 128,
                                    qstart[kt]:qstart[kt] + w])
                for qt in range(QT):
                    kts = blocks[qt]
                    pv = [pspv.tile([65, 512], f32, tag=f'pv{b}', name=f'pv{b}')
                          for b in range(B)]
                    for i, kt in enumerate(kts):
                        for b in range(B):
                            qcol = b * S + qt * 512
                            kcol = b * S + kt * 128
                            lg = pslg.tile([128, 512], f32, tag='lg')
                            nc.tensor.matmul(
                                lg[:],
                                heads_sb['k'][hb:hb + 64, kcol:kcol + 128],
                                heads_sb['q'][hb:hb + 64, qcol:qcol + 512],
                                start=True, stop=True)
                            sraw = spool.tile([128, 512], f16, tag='sraw')
                            nc.scalar.activation(sraw[:], lg[:], AF.Exp)
                            sm = spool.tile([128, 512], f16, tag='sm')
                            qoff = qt * 512 - qstart[kt]
                            nc.vector.tensor_mul(
                                sm[:], sraw[:], ebt[kt][:, qoff:qoff + 512])
                            if h == 0:
                                nc.sync.dma_start(
                                    attn_out[b, blk_index[(qt, kt)]], sm[:])
                            va = vaug[(b, kt)]
                            nc.tensor.matmul(
                                pv[b][:], va[:, 80:145] if h else va[:, 0:65],
                                sm[:], start=(i == 0), stop=(i == len(kts) - 1))
                    for b in range(B):
                        rc = rpool.tile([1, 512], f32, tag='rc')
                        nc.vector.reciprocal(rc[:], pv[b][64:65, :])
                        rcb = rpool.tile([64, 512], f32, tag='rcb')
                        nc.gpsimd.partition_broadcast(rcb[:], rc[:])
                        qcol = b * S + qt * 512
                        nc.vector.tensor_mul(
                            out_sb[hb:hb + 64, qcol:qcol + 512],
                            pv[b][0:64, :],
                            rcb[:])

            # ---- output projection partial: [E, BS] fp16 ----
            for eb in range(E // 128):
                acc = ppool.tile([128, BS], f16, tag='acc')
                for tt in range(BS // 512):
                    ps = psmm.tile([128, 512], f32, tag='mm')
                    nc.tensor.matmul(ps[:], wd_sb[:, eb * 128:(eb + 1) * 128],
                                     out_sb[:, tt * 512:(tt + 1) * 512],
                                     start=True, stop=True)
                    dst = acc[:, tt * 512:(tt + 1) * 512]
                    if tt % 2 == 0:
                        nc.scalar.copy(dst, ps[:])
                    else:
                        nc.vector.tensor_copy(dst, ps[:])
                nc.sync.dma_start(partial[eb * 128:(eb + 1) * 128, :], acc[:])
    nc.compile()
    return nc


def _prepare(q, k, v, alibi, mask, wq, wk, wv, wd):
    """Host-side prep shared across cores."""
    scale = 1.0 / np.sqrt(np.float32(D))
    xq = np.ascontiguousarray(q.reshape(BS, E).T.astype(F16))
    xk = np.ascontiguousarray(k.reshape(BS, E).T.astype(F16))
    xv = np.ascontiguousarray(v.reshape(BS, E).T.astype(F16))

    # exp(alibi + mask*(-1e9)) transposed to [H, k, q], fp16
    bias = alibi[0].astype(np.float32) + mask[0, 0].astype(np.float32) * np.float32(-1e9)
    ebT = np.ascontiguousarray(
        np.exp(bias, dtype=np.float32).swapaxes(1, 2)).astype(F16)  # [H, k, q]

    nz = (ebT.reshape(H, KT, 128, QT, 512) != 0).any(axis=(2, 4))  # [H, kt, qt]
    union = nz.any(axis=0)                                         # [kt, qt]
    blocks = {qt: [kt for kt in range(KT) if union[kt, qt]] for qt in range(QT)}
    for qt in range(QT):
        if not blocks[qt]:          # pathological all-masked column: keep one
            blocks[qt] = [min(qt * 4, KT - 1)]
    qstart, widths = {}, {}
    for kt in sorted({kt for v_ in blocks.values() for kt in v_}):
        qts = [qt for qt in range(QT) if kt in blocks[qt]]
        qstart[kt] = min(qts) * 512
        widths[kt] = (max(qts) + 1) * 512 - qstart[kt]

    per_core = []
    for c in range(NCORES):
        hA, hB = c, 15 - c
        rows = list(range(hA * D, (hA + 1) * D)) + list(range(hB * D, (hB + 1) * D))
        wq_c = np.ascontiguousarray((wq[rows, :] * scale).T.astype(F16))
        wk_c = np.ascontiguousarray(wk[rows, :].T.astype(F16))
        wv_c = np.ascontiguousarray(wv[rows, :].T.astype(F16))
        wd_c = np.ascontiguousarray(wd[:, rows].T.astype(F16))   # [128, E]
        eb_c = np.ascontiguousarray(ebT[[hA, hB]])               # [2, S, S]
        per_core.append({'wqT': wq_c, 'wkT': wk_c, 'wvT': wv_c,
                         'wdT': wd_c, 'ebias': eb_c})
    shared = {'xq': xq, 'xk': xk, 'xv': xv}
    return shared, per_core, blocks, widths, qstart


def kernel(q, k, v, alibi, mask, wq, wk, wv, wd, bd):
    from concourse import bacc
    from concourse.bass_utils import run_bass_kernel_spmd

    q, k, v = np.asarray(q), np.asarray(k), np.asarray(v)
    alibi, mask = np.asarray(alibi), np.asarray(mask)
    wq, wk, wv, wd, bd = (np.asarray(x) for x in (wq, wk, wv, wd, bd))

    shared, per_core, blocks, widths, qstart = _prepare(
        q, k, v, alibi, mask, wq, wk, wv, wd)

    key = tuple(sorted((qt, tuple(v_)) for qt, v_ in blocks.items()))
    if key not in _cache:
        nc = bacc.Bacc('TRN2', target_bir_lowering=False, debug=False,
                       num_devices=NCORES)
        _cache[key] = _build(nc, blocks, widths, qstart)
    nc = _cache[key]

    in_maps = [dict(shared, **pc) for pc in per_core]
    res = run_bass_kernel_spmd(nc, in_maps, core_ids=list(range(NCORES)))
    globals()['_last_res'] = res
    results = res.results

    # ---- host assembly ----
    out_partial = np.zeros((E, BS), np.float32)
    for r in results:
        out_partial += r['partial'].astype(np.float32)
    out = out_partial.T.reshape(B, S, E) + bd.astype(np.float32)[None, None, :]

    blk_list = [(qt, kt) for qt in range(QT) for kt in blocks[qt]]
    attn_blocks = results[0]['attn_out']                     # [B, nblk, 128, 512]
    attnT = np.zeros((B, S, S), np.float32)
    for i, (qt, kt) in enumerate(blk_list):
        attnT[:, kt * 128:(kt + 1) * 128, qt * 512:(qt + 1) * 512] = \
            attn_blocks[:, i].astype(np.float32)
    rowsum = attnT.sum(axis=1)                               # [B, q]
    rowsum[rowsum == 0.0] = 1.0
    attn = (attnT / rowsum[:, None, :]).transpose(0, 2, 1)
    return out.astype(np.float32), attn.astype(np.float32)
